# revision 27
# baseline (speedup 1.0000x reference)
"""Compact-prefix attention (nn_Attention_16234976379516) on 8 TRN2 NeuronCores.

Math per (b, h) pair:
    S = (Q @ K^T) * scale          [T, L]
    S[:, :Lc] += beta              (bias on compacted prefix)
    S = where(mask, S, -inf)       (mask folded into bias host-side)
    O = softmax(S, -1) @ V         [T, D]

Device formulation (transposed scores, no on-chip transposes):
    E^T[l, t] = exp(scale * (K Q^T)[l, t])              # PE matmul + exp
    [O*denom | denom] = sum_lc E^T_lc.T @ V'_lc         # PE PSUM accumulation
        where V'[l, :] = e^{bias[l]} * [V[l, :] | 1]    # bias folded into V on
                                                        # host; ones column gives
                                                        # the softmax denominator
    O = (O*denom) * (1/denom)                           # DVE recip + ACT scale

The exp over the 16.8M scores/core is the co-bottleneck with the PE matmuls
(both ~110us at engine roofline), so it is split across BOTH ScalarE (exact
Exp activation, 5 of 8 score tiles per head) and VectorE (3 of 8 tiles via a
one-op bf16 Schraudolph: int16(round(s*A + B)) bitcast as bf16 ~ exp(s),
applied to the non-beta l-chunks; softmax ratio cancels its mean error;
measured rel err 4.9e-3 vs the 2e-2 gate).

Sharding: (B,H) flattened to 128 pairs, 16 per core (pure H/batch split, no
cross-device communication). Host pre-transposes Q/K per head, pre-tiles V
with the ones column and e^bias (beta + mask) folded in, and casts operands
to bf16 (fp32 PSUM accumulation throughout).
"""

import numpy as np

B, H, T, L, LC, D = 4, 32, 256, 4096, 2048, 128
NCORES = 8
G = B * H                  # 128 (b,h) pairs
GPC = G // NCORES          # 16 pairs per core
N_LC = L // 128            # 32 l-chunks of 128
N_TC = T // 128            # 2 t-chunks of 128
SCALE = 1.0 / float(np.sqrt(D))

# DMA split factors (each dma_start already shards across all 16 hardware
# DMA engines; fewer splits = fewer ~650ns issue ops on the queueing engine)
KT_SPLIT = 2
V1_SPLIT = 2

_NC_CACHE = {}


def build_nc(n_heads=GPC, n_lc=N_LC, n_tc=N_TC):
    """Build the single-core Bass program (run SPMD on all 8 cores)."""
    from contextlib import ExitStack

    import concourse.bacc as bacc
    import concourse.mybir as mybir
    import concourse.tile as tile
    from concourse.bass import ts

    bf16 = mybir.dt.bfloat16
    f32 = mybir.dt.float32
    i16 = mybir.dt.int16
    L_ = n_lc * 128
    T_ = n_tc * 128
    # l-chunks of scores per wide ACT instruction (one exp over [128, ACT_W*T_])
    ACT_W = 4
    assert n_lc % ACT_W == 0
    # Score-tiles handed to DVE instead of ScalarE. DVE computes a bf16
    # Schraudolph exp in ONE tensor_scalar op: int16(round(s*A + B)) is the
    # bit pattern of bf16(~exp(s*scale)). Bias-free softmax ratio cancels the
    # approximation's mean error; placed on the non-beta (low-weight) l-chunks
    # (a >= 4 -> l >= 2048). Verified on host: rel err 7.3e-3 (gate 2e-2).
    # Engine split for exp: ACT runs ~1113ns per 4-chunk tile; DVE runs
    # ~2182ns (op + mandatory pipe DRAIN). Balance: 3 of 8 tiles on DVE.
    DVE_TILES = frozenset({4, 5, 6})
    A_TS = float(128.0 / np.log(2.0) * SCALE)
    B_TS = 16256.0 - 5.5
    # Stage-1 production order: interleave DVE-destined tiles early so the
    # slower DVE consumer starts sooner and score bufs recycle evenly.
    A_ORDER = [0, 4, 1, 5, 2, 6, 3, 7]
    # Stage-2 consumption order: ACT-produced l-chunks first, DVE-produced
    # last, giving the slower DVE maximal slack.
    LC_ORDER = (
        [lc for a in (0, 1, 2, 3, 7) for lc in range(a * ACT_W, (a + 1) * ACT_W)]
        + [lc for a in (4, 5, 6) for lc in range(a * ACT_W, (a + 1) * ACT_W)]
    )

    nc = bacc.Bacc("TRN2", target_bir_lowering=False, debug=False)
    qT_d = nc.dram_tensor("qT", [n_heads, 128, T_], bf16, kind="ExternalInput").ap()
    kT_d = nc.dram_tensor("kT", [n_heads, 128, L_], bf16, kind="ExternalInput").ap()
    # v1[g, p, lc, d]: e^bias[l] * (V row l | 1), l = lc*128 + p
    v1_d = nc.dram_tensor(
        "v1", [n_heads, 128, n_lc, 129], bf16, kind="ExternalInput"
    ).ap()
    out_d = nc.dram_tensor("out", [n_heads, n_tc, 128, 128], f32, kind="ExternalOutput").ap()

    with tile.TileContext(nc) as tc:
        with ExitStack() as ctx:
            in_pool = ctx.enter_context(tc.tile_pool(name="in_pool", bufs=3))
            e_pool = ctx.enter_context(tc.tile_pool(name="e_pool", bufs=2))
            ep_pool = ctx.enter_context(tc.tile_pool(name="ep_pool", bufs=4))
            # PSUM budget (8 banks): score tiles 2 banks x 3 bufs + two
            # single-bank O accumulators.
            s_pool = ctx.enter_context(tc.tile_pool(name="s_pool", bufs=3, space="PSUM"))
            o_pool = ctx.enter_context(tc.tile_pool(name="o_pool", bufs=2, space="PSUM"))

            def load_head(g):
                qT = in_pool.tile([128, T_], bf16, tag="qT", name="qT_sb")
                nc.sync.dma_start(out=qT, in_=qT_d[g])
                kT = in_pool.tile([128, L_], bf16, tag="kT", name="kT_sb")
                # Head 0 gates the whole pipeline: spread its kT issues over
                # otherwise-idle engine DMA queues so they don't serialize.
                kt_q = (
                    [nc.scalar, nc.sync] if g == 0 else [nc.sync] * KT_SPLIT
                )
                for c in range(KT_SPLIT):
                    w = L_ // KT_SPLIT
                    kt_q[c].dma_start(
                        out=kT[:, c * w : (c + 1) * w],
                        in_=kT_d[g, :, c * w : (c + 1) * w],
                    )
                v1 = in_pool.tile([128, n_lc, 129], bf16, tag="v1", name="v1_sb")
                for c in range(V1_SPLIT):
                    w = n_lc // V1_SPLIT
                    nc.gpsimd.dma_start(
                        out=v1[:, c * w : (c + 1) * w, :],
                        in_=v1_d[g, :, c * w : (c + 1) * w, :],
                    )
                return qT, kT, v1

            def dve_exp(e_ap, s_ap):
                nc.vector.tensor_scalar(
                    e_ap.bitcast(i16),
                    s_ap,
                    A_TS,
                    B_TS,
                    op0=mybir.AluOpType.mult,
                    op1=mybir.AluOpType.add,
                )

            def act_exp(e_ap, s_ap):
                nc.scalar.activation(
                    out=e_ap,
                    in_=s_ap,
                    func=mybir.ActivationFunctionType.Exp,
                    scale=SCALE,
                )

            def stage1_tile(qT, kT, e, a):
                """Score matmuls for l-chunks [4a, 4a+4) + exp to e (bf16)."""
                s = s_pool.tile([128, ACT_W, T_], f32, tag="s", name="s_ps")
                for j in range(ACT_W):
                    lc = a * ACT_W + j
                    nc.tensor.matmul(
                        s[:, j, :],
                        lhsT=kT[:, ts(lc, 128)],
                        rhs=qT,
                        start=True,
                        stop=True,
                    )
                e_sl = e[:, a * ACT_W : (a + 1) * ACT_W, :]
                (dve_exp if a in DVE_TILES else act_exp)(e_sl, s)

            def stage2_chunk(os_, e, v1, step):
                """Accumulate [O*denom | denom] for 4 l-chunks of LC_ORDER."""
                for i in range(step * ACT_W, (step + 1) * ACT_W):
                    lc = LC_ORDER[i]
                    for tci in range(n_tc):
                        nc.tensor.matmul(
                            os_[tci],
                            lhsT=e[:, lc, ts(tci, 128)],
                            rhs=v1[:, lc, :],
                            start=(i == 0),
                            stop=(i == n_lc - 1),
                        )

            def epilogue(os_, g):
                """O = (O*denom)/denom; recip on DVE (tiny), the wide
                scale-multiply on ScalarE via Copy with a scale AP."""
                for tci in range(n_tc):
                    recip = ep_pool.tile([128, 1], f32, tag="recip", name="recip_sb")
                    nc.vector.reciprocal(recip, os_[tci][:, 128:129])
                    ob = ep_pool.tile([128, 128], f32, tag="ob", name="ob_sb")
                    nc.scalar.activation(
                        out=ob,
                        in_=os_[tci][:, 0:128],
                        func=mybir.ActivationFunctionType.Copy,
                        scale=recip,
                    )
                    nc.sync.dma_start(out=out_d[g, tci], in_=ob)

            for g in range(n_heads):
                qT, kT, v1 = load_head(g)
                e = e_pool.tile([128, n_lc, T_], bf16, tag="e", name="e_sb")
                for a in A_ORDER:
                    stage1_tile(qT, kT, e, a)

                os_ = [
                    o_pool.tile([128, 129], f32, tag="o", name="o_ps")
                    for _ in range(n_tc)
                ]
                for step in range(len(A_ORDER)):
                    stage2_chunk(os_, e, v1, step)
                epilogue(os_, g)

    nc.compile()
    return nc


def make_core_inputs(q, k, v, beta, attn_mask):
    """Host prep: fold mask+beta into bias, transpose/tile/cast, shard 8 ways.

    Returns list of 8 in_maps (one per core)."""
    import ml_dtypes

    bf16 = ml_dtypes.bfloat16

    qf = np.ascontiguousarray(q, np.float32).reshape(G, T, D)
    kf = np.ascontiguousarray(k, np.float32).reshape(G, L, D)
    vf = np.ascontiguousarray(v, np.float32).reshape(G, L, D)

    bias = np.zeros((G, L), np.float32)
    bias[:, :LC] = np.asarray(beta, np.float32).reshape(G, LC)
    mask = np.asarray(attn_mask).reshape(G, L)
    # exp(s + b) = exp(s) * e^b: fold e^bias into the [V | 1] operand so the
    # device exp needs no per-partition bias (enables wide ACT tiles). A
    # masked-out l gets e^-inf = 0, zeroing its numerator+denominator terms.
    ebias = np.where(mask, np.exp(bias), np.float32(0.0))

    in_maps = []
    for i in range(NCORES):
        sl = slice(i * GPC, (i + 1) * GPC)
        qT = np.ascontiguousarray(qf[sl].transpose(0, 2, 1)).astype(bf16)
        kT = np.ascontiguousarray(kf[sl].transpose(0, 2, 1)).astype(bf16)
        v1 = np.empty((GPC, L, D + 1), np.float32)
        v1[..., :D] = vf[sl]
        v1[..., D] = 1.0
        v1 *= ebias[sl, :, None]
        v1 = v1.reshape(GPC, N_LC, 128, D + 1).transpose(0, 2, 1, 3)
        in_maps.append(
            {"qT": qT, "kT": kT, "v1": np.ascontiguousarray(v1.astype(bf16))}
        )
    return in_maps


def run_spmd(in_maps, trace=False):
    from concourse import bass_utils

    if "nc" not in _NC_CACHE:
        _NC_CACHE["nc"] = build_nc()
    nc = _NC_CACHE["nc"]
    return bass_utils.run_bass_kernel_spmd(
        nc, in_maps, core_ids=list(range(NCORES)), trace=trace
    )


def kernel(q, k, v, beta, attn_mask):
    res = run_spmd(make_core_inputs(q, k, v, beta, attn_mask))
    out = np.empty((G, T, D), np.float32)
    for i in range(NCORES):
        out[i * GPC : (i + 1) * GPC] = res.results[i]["out"].reshape(GPC, T, D)
    return out.reshape(B, H, T, D)

